# revision 84
# baseline (speedup 1.0000x reference)
"""Single-head causal attention (B=4, S=4096, Dm=512, Dh=64) on 8 trn2 cores.

Sharding: 8 cores = 4 batches x 2 roles. Both roles process all 4096 queries of
their batch; the causal key-tiles (128 keys each) are split mod-4: role 0 takes
tiles {0,1} mod 4, role 1 takes {2,3} mod 4. Work per core is identical in
shape (SPMD-friendly); only the data differs. Host packs each core's key
columns contiguously, and combines partial (unnormalized) outputs +
denominators at the end (max-free softmax => partials are additive).

All matmul operands are bf16 (host converts; halves DMA bytes vs fp32,
removes the fp32r small-matmul penalty on the V projection, 2x DVE
throughput on masks). PSUM accumulation stays fp32. DRAM tensors are
[128, NCH, cols] so each q-block / k,v-tranche loads with a single DMA
descriptor instead of 4. The V projection is a pure matmul into one shared
PSUM bank (4 sub-tiles packed); its bias is folded into the host combine
(out = num/den + bv), and the ones-column that accumulates the softmax
denominator comes from a one-time memset.

Device pipeline per q-block (512 queries):
  Q^T = Wq^T-chunks @ q_in^T-chunks (bf16 matmuls, PSUM fp32 accum) + bias
  per key-tile group (2 tiles): S^T[keys,q] = K^T-slice.T @ Q^T  (PSUM)
  P^T = exp(S^T * 1/8)  (one ACT call per group, PSUM->SBUF, bf16 out)
  diagonal tiles: P^T *= mask (DVE, bf16 2x)
  O^T[65,q] += V_aug-tile.T @ P^T-slice  (V_aug col 64 == 1 => row 64
  accumulates the softmax denominator)
"""

import os
import sys

sys.path.insert(0, "/opt/trn_rl_repo")

import numpy as np
import ml_dtypes

import concourse.bass as bass  # noqa: F401  (registers things)
import concourse.mybir as mybir
import concourse.tile as tile
from concourse import bacc
from concourse import bass_utils

B, S, DM, DH = 4, 4096, 512, 64
QB = 512               # queries per block
NQB = S // QB          # 8 blocks
KT = 128               # keys per tile
LOCAL_KT = 16          # key tiles per core (S / KT / 2)
LOCAL_K = LOCAL_KT * KT  # 2048 local key columns
N_CORES = 8
GROUP = 2              # key tiles per scores/exp group (PSUM banks)
NCH = DM // KT         # 4 contraction chunks
WCOLS = 5 * DH + 2     # packed weight columns (Wq x2, Wk x2, Wv+ones+pad)

FP32 = mybir.dt.float32
BF16 = mybir.dt.bfloat16
NP_BF16 = ml_dtypes.bfloat16

_CACHE = {}


def _build_program():
    nc = bacc.Bacc("TRN2", target_bir_lowering=False, debug=False,
                   num_devices=N_CORES)

    qT_d = nc.dram_tensor("qT", [KT, NCH, S], BF16, kind="ExternalInput")
    kT_d = nc.dram_tensor("kT", [KT, NCH, LOCAL_K], BF16, kind="ExternalInput")
    vT_d = nc.dram_tensor("vT", [KT, NCH, LOCAL_K], BF16, kind="ExternalInput")
    wT_d = nc.dram_tensor("wT", [KT, NCH, WCOLS], BF16, kind="ExternalInput")
    bqk_d = nc.dram_tensor("bqk", [2 * DH, 2], FP32, kind="ExternalInput")
    mask_d = nc.dram_tensor("mask", [KT, 2 * QB], BF16, kind="ExternalInput")
    oT_d = nc.dram_tensor("oT", [DH + 2, S], FP32, kind="ExternalOutput")

    with tile.TileContext(nc) as tc:
        with tc.tile_pool(name="persist", bufs=1) as persist, \
             tc.tile_pool(name="stage", bufs=4) as stage, \
             tc.tile_pool(name="qstage", bufs=8) as qstage, \
             tc.tile_pool(name="qt", bufs=4) as qtp, \
             tc.tile_pool(name="pt", bufs=6) as ptp, \
             tc.tile_pool(name="osb", bufs=3) as osbp, \
             tc.tile_pool(name="ps_proj", bufs=2, space="PSUM") as ps_proj, \
             tc.tile_pool(name="ps_scores", bufs=2, space="PSUM") as ps_scores, \
             tc.tile_pool(name="ps_oacc", bufs=2, space="PSUM") as ps_oacc:

            # ---- PE ramp warmup: keep PE busy until k0 lands so real
            # matmuls start at full p-state ----
            warm = persist.tile([KT, KT], BF16, tag="warm")
            nc.vector.memset(warm[:], 0.0)
            ps_w = ps_proj.tile([KT, KT], FP32, tag="pp")
            for _ in range(20):
                nc.tensor.matmul(ps_w[:], warm[:], warm[:],
                                 start=True, stop=True)

            # ---- constants ----
            w_sb = persist.tile([KT, NCH, WCOLS], BF16, tag="w")
            nc.sync.dma_start(out=w_sb[:], in_=wT_d.ap())
            bqk_sb = persist.tile([2 * DH, 2], FP32, tag="bqk")
            nc.gpsimd.dma_start(out=bqk_sb[:], in_=bqk_d.ap())
            mask_sb = persist.tile([KT, 2 * QB], BF16, tag="mask")

            # ---- software-pipelined tranches ----
            kt_b = []
            v_b = []
            q_stash = {}

            qt_stash = {}
            ops_stash = {}

            def attention(qb, t_lo=0, t_hi=None, close=True, rev=False):
                ntk = 2 * (qb + 1)
                if t_hi is None:
                    t_hi = ntk
                if t_lo == 0:
                    ps_q = ps_proj.tile([2 * DH, QB], FP32, tag="pp")
                    for c in range(NCH):
                        nc.tensor.matmul(ps_q[:], w_sb[:, c, 0:2 * DH],
                                         q_stash[qb][:, c, :],
                                         start=(c == 0), stop=(c == NCH - 1))
                    qt_sb = qtp.tile([2 * DH, QB], BF16, tag="qt")
                    nc.vector.tensor_scalar_add(out=qt_sb[:], in0=ps_q[:],
                                                scalar1=bqk_sb[:, 0:1])
                    qt_stash[qb] = qt_sb
                    o_new = ps_oacc.tile([DH + 2, QB], FP32, tag="oacc")
                    ops_stash[qb] = o_new
                qt_sb = qt_stash[qb]
                o_ps = ops_stash[qb]
                cnt = t_hi - t_lo
                sizes = [GROUP] * (cnt // GROUP)
                if cnt % GROUP:
                    sizes.append(cnt % GROUP)
                starts = []
                t0 = t_lo
                for glen in sizes:
                    starts.append((t0, glen))
                    t0 += glen
                if rev:
                    # diagonal group first: its mask-multiply latency hides
                    # behind the other groups instead of closing the block
                    starts.reverse()
                n_av = t_lo
                for t0, glen in starts:
                    ps_s = ps_scores.tile([KT, GROUP, QB], FP32, tag="sc")
                    for i in range(glen):
                        t = t0 + i
                        half = t % 2  # PE row-group: even->0:64, odd->64:128
                        nc.tensor.matmul(
                            ps_s[:, i, :],
                            kt_b[t // 4][half * DH:(half + 1) * DH,
                                         (t % 4) * KT:(t % 4 + 1) * KT],
                            qt_sb[half * DH:(half + 1) * DH, :],
                            start=True, stop=True)
                    pt = ptp.tile([KT, GROUP, QB], BF16, tag="pt")
                    nc.scalar.activation(
                        out=pt[:, 0:glen, :], in_=ps_s[:, 0:glen, :],
                        func=mybir.ActivationFunctionType.Exp, scale=0.125)
                    for i in range(glen):
                        t = t0 + i
                        if t >= ntk - 2:
                            m = t - (ntk - 2)
                            nc.vector.tensor_mul(
                                out=pt[:, i, :], in0=pt[:, i, :],
                                in1=mask_sb[:, m * QB:(m + 1) * QB])
                    for i in range(glen):
                        t = t0 + i
                        nc.tensor.matmul(
                            o_ps[:], v_b[t // 4][:, t % 4, :], pt[:, i, :],
                            start=(n_av == 0), stop=(n_av == ntk - 1))
                        n_av += 1
                if close:
                    o_sb = osbp.tile([DH + 2, QB], FP32, tag="osb")
                    nc.vector.tensor_copy(out=o_sb[:], in_=o_ps[:])
                    nc.sync.dma_start(
                        out=oT_d.ap()[:, qb * QB:(qb + 1) * QB], in_=o_sb[:])

            def load_q(qb):
                q_stage = qstage.tile([KT, NCH, QB], BF16, tag="qst")
                nc.sync.dma_start(
                    out=q_stage[:],
                    in_=qT_d.ap()[:, :, qb * QB:(qb + 1) * QB])
                q_stash[qb] = q_stage

            for tr in range(4):
                k_stage = stage.tile([KT, NCH, QB], BF16, tag="kst")
                v_stage = stage.tile([KT, NCH, QB], BF16, tag="vst")
                nc.sync.dma_start(
                    out=k_stage[:],
                    in_=kT_d.ap()[:, :, tr * QB:(tr + 1) * QB])
                if tr == 0:
                    load_q(0)
                    nc.sync.dma_start(out=mask_sb[:], in_=mask_d.ap())
                nc.sync.dma_start(
                    out=v_stage[:],
                    in_=vT_d.ap()[:, :, tr * QB:(tr + 1) * QB])
                if tr == 0:
                    load_q(1)
                elif tr < 3:
                    load_q(2 * tr)
                    load_q(2 * tr + 1)
                if tr == 2:
                    load_q(6)
                    load_q(7)

                if tr > 0:
                    attention(2 * (tr - 1))
                    attention(2 * (tr - 1) + 1)
                if tr == 3:
                    attention(6, 0, 12, close=False)
                    attention(7, 0, 12, close=False)

                # K^T projection (weights carry the partition-64 replica)
                kt_t = persist.tile([2 * DH, QB], BF16, tag=f"ktb{tr}")
                ps_k = ps_proj.tile([2 * DH, QB], FP32, tag="pp")
                for c in range(NCH):
                    nc.tensor.matmul(ps_k[:], w_sb[:, c, 2 * DH:4 * DH],
                                     k_stage[:, c, :],
                                     start=(c == 0), stop=(c == NCH - 1))
                nc.vector.tensor_scalar_add(out=kt_t[:], in0=ps_k[:],
                                            scalar1=bqk_sb[:, 1:2])
                kt_b.append(kt_t)
                # V projection: 4 tiles of 128 keys, one shared PSUM bank;
                # bias folded into the host combine. col 64 == 1 (denominator
                # row), col 65 == 1 (unused, kept finite)
                v_t = persist.tile([KT, QB // KT, DH + 2], BF16, tag=f"vb{tr}")
                nc.vector.memset(v_t[:, :, DH:DH + 2], 1.0)
                ps_v = ps_proj.tile([KT, QB // KT, DH], FP32, tag="pp")
                for sub in range(QB // KT):
                    for c in range(NCH):
                        nc.tensor.matmul(
                            ps_v[:, sub, :],
                            v_stage[:, c, sub * KT:(sub + 1) * KT],
                            w_sb[:, c, 4 * DH:5 * DH],
                            start=(c == 0), stop=(c == NCH - 1))
                nc.vector.tensor_copy(out=v_t[:, :, 0:DH], in_=ps_v[:])
                v_b.append(v_t)

            attention(6, 12, 14)
            attention(7, 12, 16)

    nc.compile()
    return nc


def _pack_chunks(a):
    """[DM, cols] -> [KT, NCH, cols] with row (c*KT+p) -> [p, c]."""
    cols = a.shape[1]
    return np.ascontiguousarray(
        a.reshape(NCH, KT, cols).transpose(1, 0, 2))


def _prep_inputs(q_in, k_in, v_in, Wq, bq, Wk, bk, Wv, bv):
    """Build the 8 per-core input maps (host-side, not timed)."""
    wT = np.concatenate(
        [Wq.T, Wq.T, Wk.T, Wk.T, Wv.T, np.zeros((DM, 2), np.float32)],
        axis=1).astype(np.float32)
    wT_p = _pack_chunks(wT).astype(NP_BF16)
    bqk = np.ascontiguousarray(np.stack(
        [np.concatenate([bq, bq]), np.concatenate([bk, bk])],
        axis=1)).astype(np.float32)

    # masks: mask_m[i, j] = 1 if j >= m*128 + i  (m = 2r, 2r+1)
    ii = np.arange(KT)[:, None]
    jj = np.arange(QB)[None, :]
    masks = {}
    for r in range(2):
        m0 = (jj >= (2 * r) * KT + ii).astype(NP_BF16)
        m1 = (jj >= (2 * r + 1) * KT + ii).astype(NP_BF16)
        masks[r] = np.ascontiguousarray(np.concatenate([m0, m1], axis=1))

    # per-role local key-column index sets (mod-4 tile split)
    col_idx = {}
    for r in range(2):
        idx = []
        for t in range(S // KT // 4):  # 8 super-tiles of 4
            g0 = 4 * t + 2 * r
            idx.append(np.arange(g0 * KT, (g0 + 2) * KT))
        col_idx[r] = np.concatenate(idx)

    in_maps = []
    for b in range(B):
        qT_p = _pack_chunks(q_in[b].T.astype(np.float32)).astype(NP_BF16)
        kT_full = k_in[b].T.astype(np.float32)
        vT_full = v_in[b].T.astype(np.float32)
        for r in range(2):
            in_maps.append({
                "qT": qT_p,
                "kT": _pack_chunks(kT_full[:, col_idx[r]]).astype(NP_BF16),
                "vT": _pack_chunks(vT_full[:, col_idx[r]]).astype(NP_BF16),
                "wT": wT_p,
                "bqk": bqk,
                "mask": masks[r],
            })
    return in_maps


def run_on_cores(inputs, trace=False, trace_kwargs=None):
    """Compile (cached), run on the 8 cores, return BassKernelResults."""
    if "nc" not in _CACHE:
        _CACHE["nc"] = _build_program()
    nc = _CACHE["nc"]
    in_maps = _prep_inputs(**inputs)
    res = bass_utils.run_bass_kernel_spmd(
        nc, in_maps, core_ids=list(range(N_CORES)), trace=trace,
        trace_kwargs=trace_kwargs or {})
    return res


def _combine(results, bv):
    out = np.empty((B, S, DH), dtype=np.float32)
    for b in range(B):
        o0 = results[2 * b]["oT"]
        o1 = results[2 * b + 1]["oT"]
        num = o0[:DH].astype(np.float64) + o1[:DH]
        den = o0[DH].astype(np.float64) + o1[DH]
        out[b] = (num / den + bv[:, None].astype(np.float64)).T.astype(
            np.float32)
    return out


def kernel(**inputs):
    res = run_on_cores(inputs)
    return _combine(res.results, np.asarray(inputs["bv"], np.float32))
